# revision 50
# baseline (speedup 1.0000x reference)
"""Local (windowed causal) attention Trainium2 kernel, fp8-DoubleRow edition.

Problem: B=4, L=4096, D=1024, H=16 heads, dh=64, window W=128, causal
within each window. y = OutProj(Attn(QKV(x))).

Sharding: tokens are flattened to [16384, 1024] and split across 8
cores (2048 tokens = 16 complete windows per core). Fully data
parallel; weights are broadcast. No cross-core communication.

Precision scheme (validated vs the fp32 reference in numpy):
  - x is transposed + split on host into xT8 (fp8e4m3) and its e5m2
    residual dxT5; all projection weights likewise split into an e4m3
    main part and an e5m2 residual. e5m2 for residuals matters: the
    residual magnitudes sit below e4m3's subnormal floor.
  - q,k = x8@W8 + x8@dW5                  (2 chains; score error ~0.5%)
  - v   = x8@Wv8 + x8@dWv5 + dx5@Wv8      (3 chains; window-start rows
          of the causal softmax do not attenuate v errors)
  - out = a8@Wo8 + a8@dWo5 + da5@Wo8      (3 chains; a8 = fp8(aoT),
          da5 = e5m2 residual computed on device)
  All chains are fp8 DoubleRow matmuls (2 k-tiles of contraction per
  instruction) accumulating in one PSUM group, so no extra vector ops.

Schedule: one flat software pipeline over tasks n = wb*8 + hp (window
block x head pair).  Each slot JIT-projects the NEXT task's q/k feature
tiles (fp8 DoubleRow, weights resident), then runs scores+softmax of
its own task; PE filler while the exp/mask/normalize chain drains comes
from V-projection PSUM groups (staggered so each lands just before the
aoT that reads it) and the out-proj groups of task n-9 (strictly after
that block's last aoT residual is written).  The out-proj bias and the
V bias (which passes through softmax unchanged since rows sum to 1) are
folded into a single host-side bias add.  qkT staging is double-buffered
per block; y is written straight from the out-proj PSUM via ACT copies.
"""

import numpy as np

import concourse.bass as bass
import concourse.mybir as mybir
import concourse.tile as tile
from concourse.bass_utils import run_bass_kernel_spmd
from concourse.vector_clock import ScopedClock, VectorClock

# ---------------------------------------------------------------------------
# Workaround: the pinned walrus rejects any sync-wait on an SP-engine CTRL
# (drain) instruction ("Too many sync wait commands"). Emit the end-of-kernel
# global-clock waits on non-SP engine drains instead, one wait per drain.
# ---------------------------------------------------------------------------


def _drain_and_barrier_split(self, tick_clock, wait_clock):
    g = tick_clock.global_clock
    engines = [self.nc.scalar, self.nc.vector, self.nc.gpsimd, self.nc.tensor]
    for p, t in enumerate(list(g)):
        if t == 0:
            continue
        part = VectorClock()
        part.require_at_least(p, t)
        d = engines[p % len(engines)].drain()
        wait_clock.add_sem_waits(d.ins, ScopedClock({None: part}))
    self.nc.sync.drain()
    self.nc.all_engine_barrier()
    assert self.sems is not None
    popped = self.nc._tile_sem_poison_stack.pop()
    assert popped is self._sem_poison
    self.nc.clear_and_free_semaphores(list(self.sems.allocated().values()))
    self.nc.all_engine_barrier()


tile.TileContext._drain_and_barrier = _drain_and_barrier_split


def _split_waits(nc, cap=1):
    """Hoist excess sync-waits onto standalone EventSemaphore instructions.

    The pinned walrus rejects instructions carrying more than one sync-wait
    command ("Too many sync wait commands"). Keep at most `cap` waits on each
    instruction and emit the rest as dedicated same-engine wait instructions
    immediately before it.
    """
    n = 0
    for f in nc.m.functions:
        for blk in f.blocks:
            out = []
            for inst in blk.instructions:
                si = inst.sync_info
                waits = list(si.on_wait) if si is not None and si.on_wait else []
                if len(waits) > cap:
                    keep = waits[-cap:] if cap else []
                    for wv in waits[: len(waits) - cap]:
                        n += 1
                        ev = mybir.InstEventSemaphore(
                            name=f"wsplit-{n}",
                            opcode="EventSemaphore",
                            engine=inst.engine,
                            debug=inst.debug,
                            ins=[],
                            outs=[],
                            descendants=None,
                            sync_info=mybir.SyncInfo(on_wait=[wv], on_update=[]),
                            bass_sim_breakpoint=False,
                            bass_priority=None,
                            bass_wait_until_ts=None,
                            bass_scheduled_tick=None,
                            bass_scheduled_proc=None,
                            bass_scheduled_scope=None,
                            bass_addl_debug=None,
                            bass_nofuse=True,
                        )
                        out.append(ev)
                    inst.sync_info = mybir.SyncInfo(
                        on_wait=keep, on_update=list(si.on_update)
                    )
                out.append(inst)
            blk.instructions[:] = out
    return n


# ---------------------------------------------------------------------------
# Shapes (hardcoded per spec)
# ---------------------------------------------------------------------------
B, L, D = 4, 4096, 1024
H, W = 16, 128
DH = D // H  # 64
N_CORES = 8
T = (B * L) // N_CORES  # 2048 tokens per core
NW = T // W  # 16 windows per core
KT = D // 128  # 8 k-tiles
KP = KT // 2  # 4 k-tile pairs (DoubleRow)
NF3 = 3 * D // 128  # 24 feature tiles of qkv
NQK = 2 * D // 128  # 16 feature tiles of q,k
TC = 512  # token chunk for the qk matmul
SCALE = DH**-0.5  # 0.125

F32 = mybir.dt.float32
F32R = mybir.dt.float32r
BF16 = mybir.dt.bfloat16
F8E4 = mybir.dt.float8e4
F8E5 = mybir.dt.float8e5

DR = mybir.MatmulPerfMode.DoubleRow


def build_nc(split_waits=True):
    nc = bass.Bass()

    # xT8[p, kt, t] = fp8e4(x[t, kt*128 + p]); dxT5 its e5m2 residual
    x8_in = nc.declare_dram_parameter("x8", [128, KT, T], F8E4, isOutput=False)
    dx5_in = nc.declare_dram_parameter("dx5", [128, KT, T], F8E5, isOutput=False)
    # w38[p, ft, j, i, c] = qkv_w[ft*128 + c, (2j+i)*128 + p]  (q,k rows)
    w38_in = nc.declare_dram_parameter("w38", [128, NQK, KP, 2, 128], F8E4, isOutput=False)
    dw35_in = nc.declare_dram_parameter("dw35", [128, NQK, KP, 2, 128], F8E5, isOutput=False)
    # wv8[p, j, i, f] = qkv_w[2D + f, (2j+i)*128 + p]  (v rows, moving operand)
    wv8_in = nc.declare_dram_parameter("wv8", [128, KP, 2, D], F8E4, isOutput=False)
    dwv5_in = nc.declare_dram_parameter("dwv5", [128, KP, 2, D], F8E5, isOutput=False)
    # wo8[p, j, i, f] = out_w[f, (2j+i)*128 + p]
    wo8_in = nc.declare_dram_parameter("wo8", [128, KP, 2, D], F8E4, isOutput=False)
    dwo5_in = nc.declare_dram_parameter("dwo5", [128, KP, 2, D], F8E5, isOutput=False)
    # b3 as [128, 24]: b3p[p, a] = qkv_b[a*128 + p]; q part pre-scaled
    b3_in = nc.declare_dram_parameter("b3", [128, NF3], F32, isOutput=False)
    bo_in = nc.declare_dram_parameter("bo", [D], F32, isOutput=False)
    y_out = nc.declare_dram_parameter("y", [T, D], F32, isOutput=True)

    with tile.TileContext(nc) as tc:
        with (
            tc.tile_pool(name="consts", bufs=1) as consts,
            tc.tile_pool(name="qkvt_res", bufs=1) as qkvt_pool,
            tc.tile_pool(name="x8_res", bufs=1) as x8_pool,
        ):
            # --- constants (built on device, no DMA) ---
            from concourse.masks import make_identity

            id_bf16 = consts.tile([128, 128], BF16)
            make_identity(nc, id_bf16)
            # b3 as [128, 24] (per-partition bias for qkvT feature tiles)
            b3_sb = consts.tile([128, NF3], F32)
            nc.gpsimd.dma_start(out=b3_sb, in_=b3_in[:])

            # --- qkT staging [128, 2 x 512 tokens] bf16, double-buffered by wb ---
            qkvt = [qkvt_pool.tile([128, 2, TC], BF16, name=f"qkvt{ft}") for ft in range(NQK)]
            # --- resident V in natural layout [2048 tokens, 1024 features] bf16 ---
            vnat = [qkvt_pool.tile([128, D], BF16, name=f"vnat{tt}") for tt in range(T // 128)]
            # --- resident transposed fp8 x (moving for qk, stationary for v) ---
            x8 = x8_pool.tile([128, KT, T], F8E4, name="x8")
            dx5 = x8_pool.tile([128, KT, T], F8E5, name="dx5")

            nc.sync.dma_start(out=x8[:, :, 0:512], in_=x8_in[:, :, 0:512])

            # resident q,k weights; f0/f8 ride the ACT HWDGE queue (fast
            # start), the rest go behind wv8/dwv5 on the SWDGE queue
            w38_t = [
                qkvt_pool.tile([128, KP, 2, 128], F8E4, name=f"w38_{ft}")
                for ft in range(NQK)
            ]
            dw35_t = [
                qkvt_pool.tile([128, KP, 2, 128], F8E5, name=f"dw35_{ft}")
                for ft in range(NQK)
            ]
            nc.scalar.dma_start(out=w38_t[0], in_=w38_in[:, 0])
            nc.scalar.dma_start(out=dw35_t[0], in_=dw35_in[:, 0])
            for ft in (KT, 1, KT + 1, 2, KT + 2):
                nc.sync.dma_start(out=w38_t[ft], in_=w38_in[:, ft])
                nc.sync.dma_start(out=dw35_t[ft], in_=dw35_in[:, ft])
            nc.sync.dma_start(out=dx5[:, :, 0:512], in_=dx5_in[:, :, 0:512])
            for c4 in range(1, T // 512):
                t0, t1 = c4 * 512, (c4 + 1) * 512
                nc.sync.dma_start(out=x8[:, :, t0:t1], in_=x8_in[:, :, t0:t1])
                nc.sync.dma_start(out=dx5[:, :, t0:t1], in_=dx5_in[:, :, t0:t1])

            # ============ QK projection (2 fp8 chains) for all tokens ============
            # Per-ft weight streams ride the DVE DMA queue so they overlap
            # the x8 chunk loads on the SP queue.
            # ========= V projection + attention + skewed out-projection =========
            # V-projection PSUM groups share the out-proj's `yp` buffers: V
            # groups for windows 4..15 act as PE filler during block 0's
            # attention (out-proj of block wb-1 fills blocks 1..3).
            with (
                tc.tile_pool(name="wv_res", bufs=1) as wv_pool,
                tc.tile_pool(name="wot_res", bufs=1) as wot_pool,
                tc.tile_pool(name="qk_ps", bufs=2, space="PSUM") as qk_ps,
                tc.tile_pool(name="sc_ps", bufs=1, space="PSUM") as sc_ps,
                tc.tile_pool(name="pt_ps", bufs=2, space="PSUM") as pt_ps,
                tc.tile_pool(name="ao_ps", bufs=1, space="PSUM") as ao_ps,
                tc.tile_pool(name="y_ps", bufs=1, space="PSUM") as y_ps,
                tc.tile_pool(name="attn_sb", bufs=3) as attn_sb,
                tc.tile_pool(name="ao_sb", bufs=2) as ao_sb_pool,
                tc.tile_pool(name="y_sb", bufs=3) as y_sb_pool,
            ):
                wv8_sb = wv_pool.tile([128, KP, 2, D], F8E4, name="wv8_sb")
                nc.gpsimd.dma_start(out=wv8_sb, in_=wv8_in[:])
                dwv5_sb = wv_pool.tile([128, KP, 2, D], F8E5, name="dwv5_sb")
                nc.gpsimd.dma_start(out=dwv5_sb, in_=dwv5_in[:])
                for fq in range(3, KT):
                    for ft in (fq, KT + fq):
                        nc.gpsimd.dma_start(out=w38_t[ft], in_=w38_in[:, ft])
                        nc.gpsimd.dma_start(out=dw35_t[ft], in_=dw35_in[:, ft])
                wo8_sb = wot_pool.tile([128, KP, 2, D], F8E4, name="wo8_sb")
                nc.gpsimd.dma_start(out=wo8_sb, in_=wo8_in[:])
                dwo5_sb = wot_pool.tile([128, KP, 2, D], F8E5, name="dwo5_sb")
                nc.gpsimd.dma_start(out=dwo5_sb, in_=dwo5_in[:])

                HP = H // 2  # 8 head pairs == qkv k-tiles
                WB = NW // 4  # 4 window blocks

                def v_group(tt, fo):
                    f0, f1 = fo * 512, (fo + 1) * 512
                    ps = y_ps.tile([128, 512], F32, name="yp")
                    for j in range(KP):
                        x8s = x8[:, 2 * j : 2 * j + 2, tt * 128 : (tt + 1) * 128]
                        nc.tensor.matmul(
                            ps, x8s, wv8_sb[:, j, :, f0:f1],
                            start=(j == 0), stop=False, perf_mode=DR,
                        )
                    for j in range(KP):
                        x8s = x8[:, 2 * j : 2 * j + 2, tt * 128 : (tt + 1) * 128]
                        dx5s = dx5[:, 2 * j : 2 * j + 2, tt * 128 : (tt + 1) * 128]
                        nc.tensor.matmul(
                            ps, x8s, dwv5_sb[:, j, :, f0:f1],
                            start=False, stop=False, perf_mode=DR,
                        )
                        nc.tensor.matmul(
                            ps, dx5s, wv8_sb[:, j, :, f0:f1],
                            start=False, stop=(j == KP - 1), perf_mode=DR,
                        )
                    nc.scalar.activation(
                        out=vnat[tt][:, f0:f1],
                        in_=ps,
                        func=mybir.ActivationFunctionType.Identity,
                    )

                def qk_unit(ft, wb):
                    # project 512 tokens of q or k feature-tile ft (2 chains)
                    h0 = wb * TC
                    ps = qk_ps.tile([128, TC], F32, name="ps_qk")
                    for j in range(KP):
                        x8m = x8[:, 2 * j : 2 * j + 2, h0 : h0 + TC]
                        nc.tensor.matmul(
                            ps, w38_t[ft][:, j], x8m,
                            start=(j == 0), stop=False, perf_mode=DR,
                        )
                    for j in range(KP):
                        x8m = x8[:, 2 * j : 2 * j + 2, h0 : h0 + TC]
                        nc.tensor.matmul(
                            ps, dw35_t[ft][:, j], x8m,
                            start=False, stop=(j == KP - 1), perf_mode=DR,
                        )
                    nc.scalar.activation(
                        out=qkvt[ft][:, wb % 2, :],
                        in_=ps,
                        func=mybir.ActivationFunctionType.Identity,
                        bias=b3_sb[:, ft : ft + 1],
                        scale=SCALE if ft < KT else 1.0,
                    )

                def scores(wb, hp):
                    sc = [sc_ps.tile([128, 512], F32, name=f"sc{s}") for s in range(2)]
                    for i in range(4):
                        for s in range(2):
                            r0 = s * DH
                            nc.tensor.matmul(
                                sc[s][:, i * W : (i + 1) * W],
                                qkvt[hp][r0 : r0 + DH, wb % 2, i * W : (i + 1) * W],
                                qkvt[KT + hp][r0 : r0 + DH, wb % 2, i * W : (i + 1) * W],
                                start=True,
                                stop=True,
                            )
                    return sc

                def soft(sc):
                    # p = exp(scores); mask + normalize on GPSIMD/DVE
                    p = [attn_sb.tile([128, 512], BF16, name=f"p{s}") for s in range(2)]
                    sums = attn_sb.tile([128, 2, 4], F32, name="sums")
                    recip = attn_sb.tile([128, 2, 4], F32, name="recip")
                    for s in range(2):
                        nc.scalar.activation(
                            out=p[s], in_=sc[s],
                            func=mybir.ActivationFunctionType.Exp,
                        )
                        nc.gpsimd.affine_select(
                            out=p[s].rearrange("p (i k) -> p i k", i=4),
                            in_=p[s].rearrange("p (i k) -> p i k", i=4),
                            compare_op=mybir.AluOpType.is_ge,
                            fill=0.0,
                            base=0,
                            pattern=[[0, 4], [-1, W]],
                            channel_multiplier=1,
                        )
                        nc.vector.reduce_sum(
                            out=sums[:, s, :],
                            in_=p[s].rearrange("p (i k) -> p i k", i=4),
                            axis=mybir.AxisListType.X,
                        )
                    nc.vector.reciprocal(
                        out=recip.rearrange("p a b -> p (a b)"),
                        in_=sums.rearrange("p a b -> p (a b)"),
                    )
                    for s in range(2):
                        eng = nc.vector if s == 0 else nc.gpsimd
                        for i in range(4):
                            eng.tensor_scalar_mul(
                                out=p[s][:, i * W : (i + 1) * W],
                                in0=p[s][:, i * W : (i + 1) * W],
                                scalar1=recip[:, s, i : i + 1],
                            )
                    return p

                def pt_phase(p):
                    # pT = p.T per window (PE), batched into one bank/sub
                    pt_sb = []
                    for s in range(2):
                        ptp = pt_ps.tile([128, 512], BF16, name="ptp")
                        for i in range(4):
                            nc.tensor.transpose(
                                ptp[:, i * W : (i + 1) * W],
                                p[s][:, i * W : (i + 1) * W],
                                id_bf16,
                            )
                        pts = attn_sb.tile([128, 512], BF16, name=f"pt{s}")
                        nc.vector.tensor_copy(out=pts, in_=ptp)
                        pt_sb.append(pts)
                    return pt_sb

                def ao_phase(wb, hp, pt_sb, a8, da5):
                    # aoT = v.T @ pT, both heads col-packed into one bank
                    ao = ao_ps.tile([128, 512], F32, name="ao")
                    for i in range(4):
                        wt = wb * 4 + i
                        for s in range(2):
                            f0 = hp * 128 + s * DH
                            nc.tensor.matmul(
                                ao[s * DH : (s + 1) * DH, i * W : (i + 1) * W],
                                vnat[wt][:, f0 : f0 + DH],
                                pt_sb[s][:, i * W : (i + 1) * W],
                                start=True,
                                stop=True,
                                tile_position=(0, s * DH),
                            )
                    half, hh = a8[hp // 4], hp % 4
                    dhalf = da5[hp // 4]
                    nc.scalar.activation(
                        out=half[:, hh, :], in_=ao,
                        func=mybir.ActivationFunctionType.Identity,
                    )
                    nc.vector.tensor_sub(out=dhalf[:, hh, :], in0=ao, in1=half[:, hh, :])

                def outproj_part(wb, slot, a8, da5, yp, jps):
                    i, fo = slot // 2, slot % 2
                    f0, f1 = fo * 512, (fo + 1) * 512
                    for j in jps:
                        jj = 2 * j % 4
                        a8s = a8[j // 2][:, jj : jj + 2, i * W : (i + 1) * W]
                        da5s = da5[j // 2][:, jj : jj + 2, i * W : (i + 1) * W]
                        nc.tensor.matmul(
                            yp, a8s, wo8_sb[:, j, :, f0:f1],
                            start=(j == 0), stop=False, perf_mode=DR,
                            skip_group_check=True,
                        )
                        nc.tensor.matmul(
                            yp, a8s, dwo5_sb[:, j, :, f0:f1],
                            start=False, stop=False, perf_mode=DR,
                            skip_group_check=True,
                        )
                        nc.tensor.matmul(
                            yp, da5s, wo8_sb[:, j, :, f0:f1],
                            start=False, stop=(j == KP - 1), perf_mode=DR,
                            skip_group_check=True,
                        )

                def outproj_finish(wb, slot, yp):
                    i, fo = slot // 2, slot % 2
                    c0 = wb * 4 * W + i * W
                    f0, f1 = fo * 512, (fo + 1) * 512
                    ysb = y_sb_pool.tile([128, 512], F32, name="ysb")
                    nc.scalar.copy(out=ysb, in_=yp)
                    nc.sync.dma_start(out=y_out[c0 : c0 + W, f0:f1], in_=ysb)

                # Flat global pipeline over tasks n = wb*HP + hp.  Each slot
                # prefetches the NEXT task's q/k projections (JIT), then runs
                # scores+softmax of its own task; fillers are V-projection
                # groups (staggered by the fo-half each ao reads) and the
                # previous block's out-proj groups, split around the pT
                # transposes of the previous task.
                vq1 = [(tt, 0) for tt in range(4)]          # slot 0
                vq2 = [(tt, 1) for tt in range(4)]          # slots 1..4
                vq3 = [
                    (wbv * 4 + tt, fo)
                    for wbv in range(1, 4)
                    for fo in range(2)
                    for tt in range(4)
                ]                                            # slots 5..28

                # prologue q/k units at 256-token granularity so the PE
                # starts as soon as the first quarter-chunk of x8 lands
                for ft in (0, KT):
                    for h2 in range(2):
                        h0 = h2 * 256
                        ps = qk_ps.tile([128, TC], F32, name="ps_qk", bufs=2)
                        for j in range(KP):
                            x8m = x8[:, 2 * j : 2 * j + 2, h0 : h0 + 256]
                            nc.tensor.matmul(
                                ps[:, h0 : h0 + 256] if False else ps[:, 0:256],
                                w38_t[ft][:, j], x8m,
                                start=(j == 0), stop=False, perf_mode=DR,
                            )
                        for j in range(KP):
                            x8m = x8[:, 2 * j : 2 * j + 2, h0 : h0 + 256]
                            nc.tensor.matmul(
                                ps[:, 0:256],
                                dw35_t[ft][:, j], x8m,
                                start=False, stop=(j == KP - 1), perf_mode=DR,
                            )
                        nc.scalar.activation(
                            out=qkvt[ft][:, 0, h0 : h0 + 256],
                            in_=ps[:, 0:256],
                            func=mybir.ActivationFunctionType.Identity,
                            bias=b3_sb[:, ft : ft + 1],
                            scale=SCALE if ft < KT else 1.0,
                        )

                NT = WB * HP
                blocks = {}  # wb -> (a8 halves, da5 halves)
                pend = None  # (p, wb, hp) awaiting pT/aoT
                for n in range(NT + 9):
                    wb, hp = n // HP, n % HP
                    if n + 1 < NT:
                        nwb, nhp = (n + 1) // HP, (n + 1) % HP
                        qk_unit(nhp, nwb)
                        qk_unit(KT + nhp, nwb)
                    if n < NT:
                        if hp == 0:
                            a8 = [
                                ao_sb_pool.tile([128, KT // 2, 4 * W], F8E4, name=f"a8{h}")
                                for h in range(2)
                            ]
                            da5 = [
                                ao_sb_pool.tile([128, KT // 2, 4 * W], F8E5, name=f"da5{h}")
                                for h in range(2)
                            ]
                            blocks[wb] = (a8, da5)
                        p = soft(scores(wb, hp))
                    # collect this slot's fillers
                    fills_v = []
                    if n == 0:
                        fills_v = vq1
                    elif 1 <= n <= 4:
                        fills_v = [vq2[n - 1]]
                    elif 5 <= n <= 28 and vq3:
                        fills_v = [vq3.pop(0)]
                    # out-proj group (w, s) runs at slot 8w + 9 + s, strictly
                    # after ao_phase(w, 7) which lands at slot 8w + 8
                    m = n - 9
                    yp = None
                    if m >= 0:
                        ow, os_ = m // HP, m % HP
                        pa8, pda5 = blocks[ow]
                        if n > NT:
                            # pure-tail slots: rotate through the idle qk
                            # banks as well so back-to-back groups pipeline
                            yp = qk_ps.tile([128, 512], F32, name="ps_qk")
                        else:
                            yp = y_ps.tile([128, 512], F32, name="yp")
                        outproj_part(ow, os_, pa8, pda5, yp, (0,))
                    for tt, fo in fills_v[: max(1, len(fills_v) // 2)]:
                        v_group(tt, fo)
                    if pend is not None:
                        pt_sb = pt_phase(pend[0])
                    if m >= 0:
                        outproj_part(ow, os_, pa8, pda5, yp, (1, 2, 3))
                        outproj_finish(ow, os_, yp)
                    for tt, fo in fills_v[max(1, len(fills_v) // 2) :]:
                        v_group(tt, fo)
                    if pend is not None:
                        pw, ph = pend[1], pend[2]
                        ba8, bda5 = blocks[pw]
                        ao_phase(pw, ph, pt_sb, ba8, bda5)
                    pend = (p, wb, hp) if n < NT else None

    if split_waits:
        _split_waits(nc)
    return nc


def prep_inputs(x, qkv_w, qkv_b, out_w, out_b):
    """Host-side prep: slice tokens per core, transpose + fp8-split."""
    x = np.ascontiguousarray(np.asarray(x, dtype=np.float32).reshape(B * L, D))
    qkv_w = np.asarray(qkv_w, dtype=np.float32)
    qkv_b = np.asarray(qkv_b, dtype=np.float32)
    out_w = np.asarray(out_w, dtype=np.float32)
    out_b = np.asarray(out_b, dtype=np.float32)

    import ml_dtypes

    E4, E5 = ml_dtypes.float8_e4m3, ml_dtypes.float8_e5m2

    def split8(a):
        a8 = a.astype(E4)
        r5 = (a - a8.astype(np.float32)).astype(E5)
        return a8, r5

    # w38[p, ft, j, i, c] = qkv_w[ft*128 + c, (2j+i)*128 + p]  (q,k rows)
    w3 = np.ascontiguousarray(
        qkv_w[: 2 * D].reshape(NQK, 128, KP, 2, 128).transpose(4, 0, 2, 3, 1)
    )
    w38, dw35 = split8(w3)
    # wv8[p, j, i, f] = qkv_w[2D + f, (2j+i)*128 + p]
    wv = np.ascontiguousarray(
        qkv_w[2 * D :].reshape(D, KP, 2, 128).transpose(3, 1, 2, 0)
    )
    wv8, dwv5 = split8(wv)
    # wo8[p, j, i, f] = out_w[f, (2j+i)*128 + p]
    wo = np.ascontiguousarray(out_w.reshape(D, KP, 2, 128).transpose(3, 1, 2, 0))
    wo8, dwo5 = split8(wo)

    b3 = qkv_b.copy()
    b3[:D] *= SCALE
    b3 = np.ascontiguousarray(b3.reshape(NF3, 128).T)

    in_maps = []
    for c in range(N_CORES):
        # xT8[p, kt, t] = x[c*T + t, kt*128 + p]
        xc = x[c * T : (c + 1) * T].reshape(T, KT, 128).transpose(2, 1, 0)
        x8, dx5 = split8(np.ascontiguousarray(xc))
        in_maps.append(
            {
                "x8": x8,
                "dx5": dx5,
                "w38": w38,
                "dw35": dw35,
                "wv8": wv8,
                "dwv5": dwv5,
                "wo8": wo8,
                "dwo5": dwo5,
                "b3": b3,
                "bo": out_b,
            }
        )
    return in_maps


_NC_CACHE = None


def kernel(x, qkv_w, qkv_b, out_w, out_b):
    global _NC_CACHE
    if _NC_CACHE is None:
        _NC_CACHE = build_nc()
    nc = _NC_CACHE
    in_maps = prep_inputs(x, qkv_w, qkv_b, out_w, out_b)
    res = run_bass_kernel_spmd(nc, in_maps, core_ids=list(range(N_CORES)))
    y = np.concatenate([res.results[c]["y"] for c in range(N_CORES)], axis=0)
    qkv_b = np.asarray(qkv_b, dtype=np.float32)
    out_w = np.asarray(out_w, dtype=np.float32)
    y += np.asarray(out_b, dtype=np.float32) + qkv_b[2 * D :] @ out_w.T
    return y.reshape(B, L, D)


# revision 53
# speedup vs baseline: 1.0111x; 1.0111x over previous
"""Local (windowed causal) attention Trainium2 kernel, fp8-DoubleRow edition.

Problem: B=4, L=4096, D=1024, H=16 heads, dh=64, window W=128, causal
within each window. y = OutProj(Attn(QKV(x))).

Sharding: tokens are flattened to [16384, 1024] and split across 8
cores (2048 tokens = 16 complete windows per core). Fully data
parallel; weights are broadcast. No cross-core communication.

Precision scheme (validated vs the fp32 reference in numpy):
  - x is transposed + split on host into xT8 (fp8e4m3) and its e5m2
    residual dxT5; all projection weights likewise split into an e4m3
    main part and an e5m2 residual. e5m2 for residuals matters: the
    residual magnitudes sit below e4m3's subnormal floor.
  - q,k = x8@W8 + x8@dW5                  (2 chains; score error ~0.5%)
  - v   = x8@Wv8 + x8@dWv5 + dx5@Wv8      (3 chains; window-start rows
          of the causal softmax do not attenuate v errors)
  - out = a8@Wo8 + a8@dWo5 + da5@Wo8      (3 chains; a8 = fp8(aoT),
          da5 = e5m2 residual computed on device)
  All chains are fp8 DoubleRow matmuls (2 k-tiles of contraction per
  instruction) accumulating in one PSUM group, so no extra vector ops.

Schedule: one flat software pipeline over tasks n = wb*8 + hp (window
block x head pair).  Each slot JIT-projects the NEXT task's q/k feature
tiles (fp8 DoubleRow, weights resident), then runs scores+softmax of
its own task; PE filler while the exp/mask/normalize chain drains comes
from V-projection PSUM groups (staggered so each lands just before the
aoT that reads it) and the out-proj groups of task n-9 (strictly after
that block's last aoT residual is written).  The out-proj bias and the
V bias (which passes through softmax unchanged since rows sum to 1) are
folded into a single host-side bias add.  qkT staging is double-buffered
per block; y is written straight from the out-proj PSUM via ACT copies.
"""

import numpy as np

import concourse.bass as bass
import concourse.mybir as mybir
import concourse.tile as tile
from concourse.bass_utils import run_bass_kernel_spmd
from concourse.vector_clock import ScopedClock, VectorClock

# ---------------------------------------------------------------------------
# Workaround: the pinned walrus rejects any sync-wait on an SP-engine CTRL
# (drain) instruction ("Too many sync wait commands"). Emit the end-of-kernel
# global-clock waits on non-SP engine drains instead, one wait per drain.
# ---------------------------------------------------------------------------


def _drain_and_barrier_split(self, tick_clock, wait_clock):
    g = tick_clock.global_clock
    engines = [self.nc.scalar, self.nc.vector, self.nc.gpsimd, self.nc.tensor]
    for p, t in enumerate(list(g)):
        if t == 0:
            continue
        part = VectorClock()
        part.require_at_least(p, t)
        d = engines[p % len(engines)].drain()
        wait_clock.add_sem_waits(d.ins, ScopedClock({None: part}))
    self.nc.sync.drain()
    self.nc.all_engine_barrier()
    assert self.sems is not None
    popped = self.nc._tile_sem_poison_stack.pop()
    assert popped is self._sem_poison
    self.nc.clear_and_free_semaphores(list(self.sems.allocated().values()))
    self.nc.all_engine_barrier()


tile.TileContext._drain_and_barrier = _drain_and_barrier_split


def _split_waits(nc, cap=1):
    """Hoist excess sync-waits onto standalone EventSemaphore instructions.

    The pinned walrus rejects instructions carrying more than one sync-wait
    command ("Too many sync wait commands"). Keep at most `cap` waits on each
    instruction and emit the rest as dedicated same-engine wait instructions
    immediately before it.
    """
    n = 0
    for f in nc.m.functions:
        for blk in f.blocks:
            out = []
            for inst in blk.instructions:
                si = inst.sync_info
                waits = list(si.on_wait) if si is not None and si.on_wait else []
                if len(waits) > cap:
                    keep = waits[-cap:] if cap else []
                    for wv in waits[: len(waits) - cap]:
                        n += 1
                        ev = mybir.InstEventSemaphore(
                            name=f"wsplit-{n}",
                            opcode="EventSemaphore",
                            engine=inst.engine,
                            debug=inst.debug,
                            ins=[],
                            outs=[],
                            descendants=None,
                            sync_info=mybir.SyncInfo(on_wait=[wv], on_update=[]),
                            bass_sim_breakpoint=False,
                            bass_priority=None,
                            bass_wait_until_ts=None,
                            bass_scheduled_tick=None,
                            bass_scheduled_proc=None,
                            bass_scheduled_scope=None,
                            bass_addl_debug=None,
                            bass_nofuse=True,
                        )
                        out.append(ev)
                    inst.sync_info = mybir.SyncInfo(
                        on_wait=keep, on_update=list(si.on_update)
                    )
                out.append(inst)
            blk.instructions[:] = out
    return n


# ---------------------------------------------------------------------------
# Shapes (hardcoded per spec)
# ---------------------------------------------------------------------------
B, L, D = 4, 4096, 1024
H, W = 16, 128
DH = D // H  # 64
N_CORES = 8
T = (B * L) // N_CORES  # 2048 tokens per core
NW = T // W  # 16 windows per core
KT = D // 128  # 8 k-tiles
KP = KT // 2  # 4 k-tile pairs (DoubleRow)
NF3 = 3 * D // 128  # 24 feature tiles of qkv
NQK = 2 * D // 128  # 16 feature tiles of q,k
TC = 512  # token chunk for the qk matmul
SCALE = DH**-0.5  # 0.125

F32 = mybir.dt.float32
F32R = mybir.dt.float32r
BF16 = mybir.dt.bfloat16
F8E4 = mybir.dt.float8e4
F8E5 = mybir.dt.float8e5

DR = mybir.MatmulPerfMode.DoubleRow


def build_nc(split_waits=True):
    nc = bass.Bass()

    # xT8[p, kt, t] = fp8e4(x[t, kt*128 + p]); dxT5 its e5m2 residual
    x8_in = nc.declare_dram_parameter("x8", [128, KT, T], F8E4, isOutput=False)
    dx5_in = nc.declare_dram_parameter("dx5", [128, KT, T], F8E5, isOutput=False)
    # w38[p, ft, j, i, c] = qkv_w[ft*128 + c, (2j+i)*128 + p]  (q,k rows)
    w38_in = nc.declare_dram_parameter("w38", [128, NQK, KP, 2, 128], F8E4, isOutput=False)
    dw35_in = nc.declare_dram_parameter("dw35", [128, NQK, KP, 2, 128], F8E5, isOutput=False)
    # wv8[p, j, i, f] = qkv_w[2D + f, (2j+i)*128 + p]  (v rows, moving operand)
    wv8_in = nc.declare_dram_parameter("wv8", [128, KP, 2, D], F8E4, isOutput=False)
    dwv5_in = nc.declare_dram_parameter("dwv5", [128, KP, 2, D], F8E5, isOutput=False)
    # wo8[p, j, i, f] = out_w[f, (2j+i)*128 + p]
    wo8_in = nc.declare_dram_parameter("wo8", [128, KP, 2, D], F8E4, isOutput=False)
    dwo5_in = nc.declare_dram_parameter("dwo5", [128, KP, 2, D], F8E5, isOutput=False)
    # b3 as [128, 24]: b3p[p, a] = qkv_b[a*128 + p]; q part pre-scaled
    b3_in = nc.declare_dram_parameter("b3", [128, NF3], F32, isOutput=False)
    bo_in = nc.declare_dram_parameter("bo", [D], F32, isOutput=False)
    y_out = nc.declare_dram_parameter("y", [T, D], F32, isOutput=True)

    with tile.TileContext(nc) as tc:
        with (
            tc.tile_pool(name="consts", bufs=1) as consts,
            tc.tile_pool(name="qkvt_res", bufs=1) as qkvt_pool,
            tc.tile_pool(name="x8_res", bufs=1) as x8_pool,
        ):
            # --- constants (built on device, no DMA) ---
            from concourse.masks import make_identity

            id_bf16 = consts.tile([128, 128], BF16)
            make_identity(nc, id_bf16)
            # b3 as [128, 24] (per-partition bias for qkvT feature tiles)
            b3_sb = consts.tile([128, NF3], F32)
            nc.gpsimd.dma_start(out=b3_sb, in_=b3_in[:])

            # --- qkT staging [128, 2 x 512 tokens] bf16, double-buffered by wb ---
            qkvt = [qkvt_pool.tile([128, 2, TC], BF16, name=f"qkvt{ft}") for ft in range(NQK)]
            # --- resident V in natural layout [2048 tokens, 1024 features] bf16 ---
            vnat = [qkvt_pool.tile([128, D], BF16, name=f"vnat{tt}") for tt in range(T // 128)]
            # --- resident transposed fp8 x (moving for qk, stationary for v) ---
            x8 = x8_pool.tile([128, KT, T], F8E4, name="x8")
            dx5 = x8_pool.tile([128, KT, T], F8E5, name="dx5")

            nc.sync.dma_start(out=x8[:, :, 0:512], in_=x8_in[:, :, 0:512])

            # resident q,k weights; f0/f8 ride the ACT HWDGE queue (fast
            # start), the rest go behind wv8/dwv5 on the SWDGE queue
            w38_t = [
                qkvt_pool.tile([128, KP, 2, 128], F8E4, name=f"w38_{ft}")
                for ft in range(NQK)
            ]
            dw35_t = [
                qkvt_pool.tile([128, KP, 2, 128], F8E5, name=f"dw35_{ft}")
                for ft in range(NQK)
            ]
            nc.scalar.dma_start(out=w38_t[0], in_=w38_in[:, 0])
            nc.scalar.dma_start(out=dw35_t[0], in_=dw35_in[:, 0])
            for ft in (KT, 1, KT + 1, 2, KT + 2):
                nc.sync.dma_start(out=w38_t[ft], in_=w38_in[:, ft])
                nc.sync.dma_start(out=dw35_t[ft], in_=dw35_in[:, ft])
            nc.sync.dma_start(out=dx5[:, :, 0:512], in_=dx5_in[:, :, 0:512])
            for c4 in range(1, T // 512):
                t0, t1 = c4 * 512, (c4 + 1) * 512
                nc.sync.dma_start(out=x8[:, :, t0:t1], in_=x8_in[:, :, t0:t1])
                nc.sync.dma_start(out=dx5[:, :, t0:t1], in_=dx5_in[:, :, t0:t1])

            # ============ QK projection (2 fp8 chains) for all tokens ============
            # Per-ft weight streams ride the DVE DMA queue so they overlap
            # the x8 chunk loads on the SP queue.
            # ========= V projection + attention + skewed out-projection =========
            # V-projection PSUM groups share the out-proj's `yp` buffers: V
            # groups for windows 4..15 act as PE filler during block 0's
            # attention (out-proj of block wb-1 fills blocks 1..3).
            with (
                tc.tile_pool(name="wv_res", bufs=1) as wv_pool,
                tc.tile_pool(name="wot_res", bufs=1) as wot_pool,
                tc.tile_pool(name="qk_ps", bufs=2, space="PSUM") as qk_ps,
                tc.tile_pool(name="sc_ps", bufs=1, space="PSUM") as sc_ps,
                tc.tile_pool(name="pt_ps", bufs=2, space="PSUM") as pt_ps,
                tc.tile_pool(name="ao_ps", bufs=1, space="PSUM") as ao_ps,
                tc.tile_pool(name="y_ps", bufs=1, space="PSUM") as y_ps,
                tc.tile_pool(name="attn_sb", bufs=3) as attn_sb,
                tc.tile_pool(name="ao_sb", bufs=2) as ao_sb_pool,
                tc.tile_pool(name="y_sb", bufs=3) as y_sb_pool,
            ):
                wv8_sb = wv_pool.tile([128, KP, 2, D], F8E4, name="wv8_sb")
                nc.gpsimd.dma_start(out=wv8_sb, in_=wv8_in[:])
                dwv5_sb = wv_pool.tile([128, KP, 2, D], F8E5, name="dwv5_sb")
                nc.gpsimd.dma_start(out=dwv5_sb, in_=dwv5_in[:])
                for fq in range(3, KT):
                    for ft in (fq, KT + fq):
                        nc.gpsimd.dma_start(out=w38_t[ft], in_=w38_in[:, ft])
                        nc.gpsimd.dma_start(out=dw35_t[ft], in_=dw35_in[:, ft])
                wo8_sb = wot_pool.tile([128, KP, 2, D], F8E4, name="wo8_sb")
                nc.gpsimd.dma_start(out=wo8_sb, in_=wo8_in[:])
                dwo5_sb = wot_pool.tile([128, KP, 2, D], F8E5, name="dwo5_sb")
                nc.gpsimd.dma_start(out=dwo5_sb, in_=dwo5_in[:])

                HP = H // 2  # 8 head pairs == qkv k-tiles
                WB = NW // 4  # 4 window blocks

                def v_group(tt, fo):
                    f0, f1 = fo * 512, (fo + 1) * 512
                    ps = y_ps.tile([128, 512], F32, name="yp")
                    for j in range(KP):
                        x8s = x8[:, 2 * j : 2 * j + 2, tt * 128 : (tt + 1) * 128]
                        nc.tensor.matmul(
                            ps, x8s, wv8_sb[:, j, :, f0:f1],
                            start=(j == 0), stop=False, perf_mode=DR,
                        )
                    for j in range(KP):
                        x8s = x8[:, 2 * j : 2 * j + 2, tt * 128 : (tt + 1) * 128]
                        dx5s = dx5[:, 2 * j : 2 * j + 2, tt * 128 : (tt + 1) * 128]
                        nc.tensor.matmul(
                            ps, x8s, dwv5_sb[:, j, :, f0:f1],
                            start=False, stop=False, perf_mode=DR,
                        )
                        nc.tensor.matmul(
                            ps, dx5s, wv8_sb[:, j, :, f0:f1],
                            start=False, stop=(j == KP - 1), perf_mode=DR,
                        )
                    nc.scalar.activation(
                        out=vnat[tt][:, f0:f1],
                        in_=ps,
                        func=mybir.ActivationFunctionType.Identity,
                    )

                def qk_unit(ft, wb):
                    # project 512 tokens of q or k feature-tile ft (2 chains)
                    h0 = wb * TC
                    ps = qk_ps.tile([128, TC], F32, name="ps_qk")
                    for j in range(KP):
                        x8m = x8[:, 2 * j : 2 * j + 2, h0 : h0 + TC]
                        nc.tensor.matmul(
                            ps, w38_t[ft][:, j], x8m,
                            start=(j == 0), stop=False, perf_mode=DR,
                        )
                    for j in range(KP):
                        x8m = x8[:, 2 * j : 2 * j + 2, h0 : h0 + TC]
                        nc.tensor.matmul(
                            ps, dw35_t[ft][:, j], x8m,
                            start=False, stop=(j == KP - 1), perf_mode=DR,
                        )
                    nc.scalar.activation(
                        out=qkvt[ft][:, wb % 2, :],
                        in_=ps,
                        func=mybir.ActivationFunctionType.Identity,
                        bias=b3_sb[:, ft : ft + 1],
                        scale=SCALE if ft < KT else 1.0,
                    )

                def scores(wb, hp):
                    sc = [sc_ps.tile([128, 512], F32, name=f"sc{s}") for s in range(2)]
                    for i in range(4):
                        for s in range(2):
                            r0 = s * DH
                            nc.tensor.matmul(
                                sc[s][:, i * W : (i + 1) * W],
                                qkvt[hp][r0 : r0 + DH, wb % 2, i * W : (i + 1) * W],
                                qkvt[KT + hp][r0 : r0 + DH, wb % 2, i * W : (i + 1) * W],
                                start=True,
                                stop=True,
                            )
                    return sc

                def soft(sc):
                    # p = exp(scores); mask + normalize on GPSIMD/DVE
                    p = [attn_sb.tile([128, 512], BF16, name=f"p{s}") for s in range(2)]
                    sums = attn_sb.tile([128, 2, 4], F32, name="sums")
                    recip = attn_sb.tile([128, 2, 4], F32, name="recip")
                    for s in range(2):
                        nc.scalar.activation(
                            out=p[s], in_=sc[s],
                            func=mybir.ActivationFunctionType.Exp,
                        )
                        nc.gpsimd.affine_select(
                            out=p[s].rearrange("p (i k) -> p i k", i=4),
                            in_=p[s].rearrange("p (i k) -> p i k", i=4),
                            compare_op=mybir.AluOpType.is_ge,
                            fill=0.0,
                            base=0,
                            pattern=[[0, 4], [-1, W]],
                            channel_multiplier=1,
                        )
                        nc.vector.reduce_sum(
                            out=sums[:, s, :],
                            in_=p[s].rearrange("p (i k) -> p i k", i=4),
                            axis=mybir.AxisListType.X,
                        )
                    nc.vector.reciprocal(
                        out=recip.rearrange("p a b -> p (a b)"),
                        in_=sums.rearrange("p a b -> p (a b)"),
                    )
                    for s in range(2):
                        eng = nc.vector if s == 0 else nc.gpsimd
                        for i in range(4):
                            eng.tensor_scalar_mul(
                                out=p[s][:, i * W : (i + 1) * W],
                                in0=p[s][:, i * W : (i + 1) * W],
                                scalar1=recip[:, s, i : i + 1],
                            )
                    return p

                def pt_phase(p):
                    # pT = p.T per window (PE), batched into one bank/sub
                    pt_sb = []
                    for s in range(2):
                        ptp = pt_ps.tile([128, 512], BF16, name="ptp")
                        for i in range(4):
                            nc.tensor.transpose(
                                ptp[:, i * W : (i + 1) * W],
                                p[s][:, i * W : (i + 1) * W],
                                id_bf16,
                            )
                        pts = attn_sb.tile([128, 512], BF16, name=f"pt{s}")
                        nc.vector.tensor_copy(out=pts, in_=ptp)
                        pt_sb.append(pts)
                    return pt_sb

                def ao_phase(wb, hp, pt_sb, a8, da5):
                    # aoT = v.T @ pT, both heads col-packed into one bank
                    ao = ao_ps.tile([128, 512], F32, name="ao")
                    for i in range(4):
                        wt = wb * 4 + i
                        for s in range(2):
                            f0 = hp * 128 + s * DH
                            nc.tensor.matmul(
                                ao[s * DH : (s + 1) * DH, i * W : (i + 1) * W],
                                vnat[wt][:, f0 : f0 + DH],
                                pt_sb[s][:, i * W : (i + 1) * W],
                                start=True,
                                stop=True,
                                tile_position=(0, s * DH),
                            )
                    half, hh = a8[hp // 4], hp % 4
                    dhalf = da5[hp // 4]
                    nc.scalar.activation(
                        out=half[:, hh, :], in_=ao,
                        func=mybir.ActivationFunctionType.Identity,
                    )
                    nc.vector.tensor_sub(out=dhalf[:, hh, :], in0=ao, in1=half[:, hh, :])

                def outproj_part(wb, slot, a8, da5, yp, jps):
                    i, fo = slot // 2, slot % 2
                    f0, f1 = fo * 512, (fo + 1) * 512
                    for j in jps:
                        jj = 2 * j % 4
                        a8s = a8[j // 2][:, jj : jj + 2, i * W : (i + 1) * W]
                        da5s = da5[j // 2][:, jj : jj + 2, i * W : (i + 1) * W]
                        nc.tensor.matmul(
                            yp, a8s, wo8_sb[:, j, :, f0:f1],
                            start=(j == 0), stop=False, perf_mode=DR,
                            skip_group_check=True,
                        )
                        nc.tensor.matmul(
                            yp, a8s, dwo5_sb[:, j, :, f0:f1],
                            start=False, stop=False, perf_mode=DR,
                            skip_group_check=True,
                        )
                        nc.tensor.matmul(
                            yp, da5s, wo8_sb[:, j, :, f0:f1],
                            start=False, stop=(j == KP - 1), perf_mode=DR,
                            skip_group_check=True,
                        )

                def outproj_finish(wb, slot, yp):
                    i, fo = slot // 2, slot % 2
                    c0 = wb * 4 * W + i * W
                    f0, f1 = fo * 512, (fo + 1) * 512
                    ysb = y_sb_pool.tile([128, 512], F32, name="ysb")
                    nc.scalar.copy(out=ysb, in_=yp)
                    nc.sync.dma_start(out=y_out[c0 : c0 + W, f0:f1], in_=ysb)

                # Flat global pipeline over tasks n = wb*HP + hp.  Each slot
                # prefetches the NEXT task's q/k projections (JIT), then runs
                # scores+softmax of its own task; fillers are V-projection
                # groups (staggered by the fo-half each ao reads) and the
                # previous block's out-proj groups, split around the pT
                # transposes of the previous task.
                vq1 = [(tt, 0) for tt in range(4)]          # slot 0
                vq2 = [(tt, 1) for tt in range(4)]          # slots 1..4
                vq3 = [
                    (wbv * 4 + tt, fo)
                    for wbv in range(1, 4)
                    for fo in range(2)
                    for tt in range(4)
                ]                                            # slots 5..28

                qk_unit(0, 0)
                qk_unit(KT, 0)

                NT = WB * HP
                blocks = {}  # wb -> (a8 halves, da5 halves)
                pend = None  # (p, wb, hp) awaiting pT/aoT
                for n in range(NT + 9):
                    wb, hp = n // HP, n % HP
                    if n + 1 < NT:
                        nwb, nhp = (n + 1) // HP, (n + 1) % HP
                        qk_unit(nhp, nwb)
                        qk_unit(KT + nhp, nwb)
                    if n < NT:
                        if hp == 0:
                            a8 = [
                                ao_sb_pool.tile([128, KT // 2, 4 * W], F8E4, name=f"a8{h}")
                                for h in range(2)
                            ]
                            da5 = [
                                ao_sb_pool.tile([128, KT // 2, 4 * W], F8E5, name=f"da5{h}")
                                for h in range(2)
                            ]
                            blocks[wb] = (a8, da5)
                        p = soft(scores(wb, hp))
                    # collect this slot's fillers
                    fills_v = []
                    if n == 0:
                        fills_v = vq1
                    elif 1 <= n <= 4:
                        fills_v = [vq2[n - 1]]
                    elif 5 <= n <= 28 and vq3:
                        fills_v = [vq3.pop(0)]
                    # out-proj group (w, s) runs at slot 8w + 9 + s, strictly
                    # after ao_phase(w, 7) which lands at slot 8w + 8
                    m = n - 9
                    yp = None
                    if m >= 0:
                        ow, os_ = m // HP, m % HP
                        pa8, pda5 = blocks[ow]
                        if n > NT:
                            # pure-tail slots: rotate through the idle qk
                            # banks as well so back-to-back groups pipeline
                            yp = qk_ps.tile([128, 512], F32, name="ps_qk")
                        else:
                            yp = y_ps.tile([128, 512], F32, name="yp")
                        outproj_part(ow, os_, pa8, pda5, yp, (0,))
                    for tt, fo in fills_v[: max(1, len(fills_v) // 2)]:
                        v_group(tt, fo)
                    if pend is not None:
                        pt_sb = pt_phase(pend[0])
                    if m >= 0:
                        outproj_part(ow, os_, pa8, pda5, yp, (1, 2, 3))
                        outproj_finish(ow, os_, yp)
                    for tt, fo in fills_v[max(1, len(fills_v) // 2) :]:
                        v_group(tt, fo)
                    if pend is not None:
                        pw, ph = pend[1], pend[2]
                        ba8, bda5 = blocks[pw]
                        ao_phase(pw, ph, pt_sb, ba8, bda5)
                    pend = (p, wb, hp) if n < NT else None

    if split_waits:
        _split_waits(nc)
    return nc


def prep_inputs(x, qkv_w, qkv_b, out_w, out_b):
    """Host-side prep: slice tokens per core, transpose + fp8-split."""
    x = np.ascontiguousarray(np.asarray(x, dtype=np.float32).reshape(B * L, D))
    qkv_w = np.asarray(qkv_w, dtype=np.float32)
    qkv_b = np.asarray(qkv_b, dtype=np.float32)
    out_w = np.asarray(out_w, dtype=np.float32)
    out_b = np.asarray(out_b, dtype=np.float32)

    import ml_dtypes

    E4, E5 = ml_dtypes.float8_e4m3, ml_dtypes.float8_e5m2

    def split8(a):
        a8 = a.astype(E4)
        r5 = (a - a8.astype(np.float32)).astype(E5)
        return a8, r5

    # w38[p, ft, j, i, c] = qkv_w[ft*128 + c, (2j+i)*128 + p]  (q,k rows)
    w3 = np.ascontiguousarray(
        qkv_w[: 2 * D].reshape(NQK, 128, KP, 2, 128).transpose(4, 0, 2, 3, 1)
    )
    w38, dw35 = split8(w3)
    # wv8[p, j, i, f] = qkv_w[2D + f, (2j+i)*128 + p]
    wv = np.ascontiguousarray(
        qkv_w[2 * D :].reshape(D, KP, 2, 128).transpose(3, 1, 2, 0)
    )
    wv8, dwv5 = split8(wv)
    # wo8[p, j, i, f] = out_w[f, (2j+i)*128 + p]
    wo = np.ascontiguousarray(out_w.reshape(D, KP, 2, 128).transpose(3, 1, 2, 0))
    wo8, dwo5 = split8(wo)

    b3 = qkv_b.copy()
    b3[:D] *= SCALE
    b3 = np.ascontiguousarray(b3.reshape(NF3, 128).T)

    in_maps = []
    for c in range(N_CORES):
        # xT8[p, kt, t] = x[c*T + t, kt*128 + p]
        xc = x[c * T : (c + 1) * T].reshape(T, KT, 128).transpose(2, 1, 0)
        x8, dx5 = split8(np.ascontiguousarray(xc))
        in_maps.append(
            {
                "x8": x8,
                "dx5": dx5,
                "w38": w38,
                "dw35": dw35,
                "wv8": wv8,
                "dwv5": dwv5,
                "wo8": wo8,
                "dwo5": dwo5,
                "b3": b3,
                "bo": out_b,
            }
        )
    return in_maps


_NC_CACHE = None


def kernel(x, qkv_w, qkv_b, out_w, out_b):
    global _NC_CACHE
    if _NC_CACHE is None:
        _NC_CACHE = build_nc()
    nc = _NC_CACHE
    in_maps = prep_inputs(x, qkv_w, qkv_b, out_w, out_b)
    res = run_bass_kernel_spmd(nc, in_maps, core_ids=list(range(N_CORES)))
    y = np.concatenate([res.results[c]["y"] for c in range(N_CORES)], axis=0)
    qkv_b = np.asarray(qkv_b, dtype=np.float32)
    out_w = np.asarray(out_w, dtype=np.float32)
    y += np.asarray(out_b, dtype=np.float32) + qkv_b[2 * D :] @ out_w.T
    return y.reshape(B, L, D)
